# revision 33
# baseline (speedup 1.0000x reference)
"""Causal multi-head attention block (QKV proj + causal softmax attention + out proj)
for Trainium2, sharded over 8 NeuronCores.

Sharding: tensor-parallel over heads x data-parallel over batch.
  core (b, g) for b in {0,1}, g in {0..3}: batch b, head group g (4 heads of 16).
  Each core computes its 4 heads' attention output slice and a partial
  output projection (row-parallel W_O); host sums the 4 partials per batch.

Device layout: everything is computed in "transposed" orientation so no
on-chip transposes are needed anywhere:
  - host passes x^T, W_qkv^T (sliced), W_O^T (sliced) per core
  - Q^T,K^T = (W^T)^T @ x^T via PE;  V in natural [t,d] layout (+ ones column)
  - S^T[k,q] = (K^T)^T @ Q^T; exp on ScalarE (no max-subtraction needed:
    scores are ~N(0,1), exp is safe in fp32); causal mask via an additive
    -1e30 triangle on the diagonal ST psum tile before exp
  - O^T[d,q] (+ sum row from the ones column) = V_aug^T @ P^T, accumulated
    only over the causally-valid column range per k-tile
  - normalize with reciprocal_approx_fast + gpsimd partition_broadcast
  - partial_out[t,o] = (attn^T)^T @ W_O^T

All matmul operands use dtype float32r (fp32 bits, PE 'replicated' datapath):
full bf16-rate matmul at ~1e-4 relative accuracy.
"""

import sys

sys.path.insert(0, "/opt/trn_rl_repo")

import numpy as np

import concourse.bacc as bacc
import concourse.mybir as mybir
import concourse.tile as tile
from concourse import bass_utils

B, T, C = 2, 2048, 1024
H, DK = 16, 64
G = 4  # tensor-parallel head groups
HG = H // G  # heads per core
WQK = 2 * HG * DK  # 512: Q+K cols per core in wqkvT
WV = HG * DK  # 256: V cols per core
N_CORES = 8
F32 = mybir.dt.float32
F32R = mybir.dt.float32r

TCH = 4  # t chunks of 512 for N-dim of matmuls
CK = C // 128  # 8 contraction chunks
NT = T // 128  # 16 t-tiles
QCH = 512  # q chunk
MASKVAL = -1e30


def _emit(nc, xT, wqkvT, woT, ones, out, taps=None):
    with tile.TileContext(nc) as tc:
        with (
            tc.tile_pool(name="persist", bufs=1) as persist,
            tc.tile_pool(name="pt", bufs=3) as pt_pool,
            tc.tile_pool(name="small", bufs=2) as small_pool,
            tc.tile_pool(name="ob", bufs=3) as ob_pool,
            tc.tile_pool(name="qkv_ps", bufs=2, space="PSUM") as qkv_ps,
            tc.tile_pool(name="st_ps", bufs=2, space="PSUM") as st_ps,
            tc.tile_pool(name="ot_ps", bufs=2, space="PSUM") as ot_ps,
        ):
            wo_ps = qkv_ps
            xT_all = persist.tile([128, CK, T], F32R, tag="xT_all")
            w_all = persist.tile([128, CK, 3 * WV], F32R, tag="w_all")
            woT_all = persist.tile([128, 2, C], F32R, tag="woT_all")
            qkT = persist.tile([128, 4, T], F32R, tag="qkT")
            vaug = persist.tile([128, NT, HG, DK + 1], F32R, tag="vaug")
            attnT = persist.tile([128, 2, T], F32R, tag="attnT")

            # weights + first x t-chunk interleaved, then remaining x t-chunks
            for k in range(CK):
                nc.sync.dma_start(
                    w_all[:, k, :], wqkvT[k * 128 : (k + 1) * 128, :]
                )
                nc.sync.dma_start(
                    xT_all[:, k, 0:QCH], xT[k * 128 : (k + 1) * 128, 0:QCH]
                )
            # ones column (last col) for the softmax-denominator rows
            nc.sync.dma_start(vaug[:, :, :, DK : DK + 1], ones[:])
            for tch in range(1, TCH):
                for k in range(CK):
                    nc.sync.dma_start(
                        xT_all[:, k, tch * QCH : (tch + 1) * QCH],
                        xT[k * 128 : (k + 1) * 128, tch * QCH : (tch + 1) * QCH],
                    )
            for j in range(2):
                nc.sync.dma_start(woT_all[:, j, :], woT[j * 128 : (j + 1) * 128, :])

            def qk_mm(ps, j, tch, k):
                nc.tensor.matmul(
                    ps[:],
                    w_all[:, k, j * 128 : (j + 1) * 128],
                    xT_all[:, k, tch * QCH : (tch + 1) * QCH],
                    start=(k == 0),
                    stop=(k == CK - 1),
                )

            def qk_copy(ps, j, tch):
                nc.vector.tensor_copy(qkT[:, j, tch * QCH : (tch + 1) * QCH], ps[:])

            def v_mm(ps, ti, k):
                nc.tensor.matmul(
                    ps[:],
                    xT_all[:, k, ti * 128 : (ti + 1) * 128],
                    w_all[:, k, WQK : WQK + WV],
                    start=(k == 0),
                    stop=(k == CK - 1),
                )

            def v_copy(ps, ti):
                nc.vector.tensor_copy(
                    vaug[:, ti, :, 0:DK],
                    ps[:].rearrange("p (h d) -> p h d", h=HG),
                )

            def emit_qk(j, tch):
                # qkT[:, j, t-chunk] = W[:, j*128:(j+1)*128].T @ xT
                ps = qkv_ps.tile([128, QCH], F32, tag="mm")
                for k in range(CK):
                    qk_mm(ps, j, tch, k)
                qk_copy(ps, j, tch)

            def emit_v(ti):
                # vaug[:, ti, h, 1:] = xT[:, ti-tile].T @ Wv  -> [128 t, 256]
                ps = qkv_ps.tile([128, WV], F32, tag="mm")
                for k in range(CK):
                    v_mm(ps, ti, k)
                v_copy(ps, ti)

            def emit_qkv_chunk0():
                # DMA-bound startup: k-outer over all four W tiles and the
                # first two V tiles so the PE consumes each arriving c-chunk
                # of x/w with ~6 matmuls instead of stalling per psum group.
                ps_j = {
                    j: qkv_ps.tile([128, QCH], F32, tag="mm", name=f"ps_j{j}")
                    for j in (0, 2)
                }
                ps_j[1] = ot_ps.tile([128, QCH], F32, tag="ot", name="ps_j1")
                ps_j[3] = ot_ps.tile([128, QCH], F32, tag="ot", name="ps_j3")
                ps_v = {
                    ti: st_ps.tile([128, WV], F32, tag="st", name=f"ps_v{ti}")
                    for ti in (0, 1)
                }
                for k in range(CK):
                    for j in (0, 2, 1, 3):
                        qk_mm(ps_j[j], j, 0, k)
                    for ti in (0, 1):
                        v_mm(ps_v[ti], ti, k)
                for j in (0, 2, 1, 3):
                    qk_copy(ps_j[j], j, 0)
                for ti in (0, 1):
                    v_copy(ps_v[ti], ti)
                for ti in (2, 3):
                    emit_v(ti)

            def emit_head_chunk(h, q0, qlen):
                prow = (h % 2) * 64
                QT_h = qkT[prow : prow + 64, h // 2, :]
                KT_h = qkT[prow : prow + 64, 2 + h // 2, :]
                if True:
                    nk = (q0 + qlen) // 128
                    n_full = max((q0 - 0) // 128, 0)  # k-tiles fully below diag
                    n_merge = n_full // 2
                    ot = ot_ps.tile([DK + 1, QCH], F32, tag="ot", name="ot")[:, 0:qlen]

                    def pv_mm(pt_ap, k):
                        nc.tensor.matmul(
                            ot[:, max(k * 128 - q0, 0) : qlen],
                            vaug[:, k, h, :],
                            pt_ap,
                            start=(k == 0),
                            stop=(k == nk - 1),
                        )

                    # fully-valid k-tiles in merged pairs: two ST matmuls into
                    # a 2-bank psum tile, ONE exp over both (amortizes the
                    # per-op ScalarE overhead), two PV accumulations.
                    for kp in range(n_merge):
                        ka, kb = 2 * kp, 2 * kp + 1
                        st2 = st_ps.tile([128, 2, QCH], F32, tag="st", name="st2")
                        for i, k in ((0, ka), (1, kb)):
                            nc.tensor.matmul(
                                st2[:, i, 0:qlen],
                                KT_h[:, k * 128 : (k + 1) * 128],
                                QT_h[:, q0 : q0 + qlen],
                                start=True,
                                stop=True,
                            )
                        pt2 = pt_pool.tile([128, 2, QCH], F32R, tag="pt", name="pt2")
                        nc.scalar.activation(
                            pt2[:, :, 0:qlen], st2[:, :, 0:qlen],
                            mybir.ActivationFunctionType.Exp,
                            scale=float(1.0 / np.sqrt(DK)),
                        )
                        pv_mm(pt2[:, 0, 0:qlen], ka)
                        pv_mm(pt2[:, 1, 0:qlen], kb)

                    for k in range(2 * n_merge, nk):
                        k0 = k * 128
                        delta = k0 - q0
                        d0 = max(delta, 0)
                        st = st_ps.tile([128, 2, QCH], F32, tag="st", name="st")[:, 0, 0:qlen]
                        nc.tensor.matmul(
                            st[:, d0:qlen],
                            KT_h[:, k0 : k0 + 128],
                            QT_h[:, q0 + d0 : q0 + qlen],
                            start=True,
                            stop=True,
                        )
                        pt = pt_pool.tile([128, 2, QCH], F32R, tag="pt", name="pt")[:, 0, 0:qlen]
                        nc.scalar.activation(
                            pt[:, d0:qlen], st[:, d0:qlen],
                            mybir.ActivationFunctionType.Exp,
                            scale=float(1.0 / np.sqrt(DK)),
                        )
                        if delta >= 0:
                            # diagonal tile: zero entries with q_global < k_global
                            nc.gpsimd.affine_select(
                                out=pt[:, d0 : d0 + 128],
                                in_=pt[:, d0 : d0 + 128],
                                compare_op=mybir.AluOpType.is_ge,
                                fill=0.0,
                                base=0,
                                pattern=[[1, 128]],
                                channel_multiplier=-1,
                            )
                        # accumulate only the causally-valid columns; columns
                        # < d0 get no contribution from this k-tile (they are
                        # exactly zero), and k=0 (d0=0) initializes the bank.
                        pv_mm(pt[:, d0:qlen], k)
                    # custom-DVE recip needs a base-partition-0 source; stage
                    # the sums row (psum partition 64) through sbuf partition 0
                    sums_sb = small_pool.tile([1, QCH], F32, tag="sums", name="sums_sb")[:, 0:qlen]
                    nc.vector.tensor_copy(sums_sb[:], ot[DK : DK + 1, :])
                    recip = small_pool.tile([1, QCH], F32, tag="recip", name="recip")[:, 0:qlen]
                    nc.vector.reciprocal_approx_fast(out=recip[:], in_=sums_sb[:])
                    rb = small_pool.tile([64, QCH], F32, tag="rb", name="rb")[:, 0:qlen]
                    nc.gpsimd.partition_broadcast(rb[:], recip[:])
                    nc.vector.tensor_tensor(
                        attnT[prow : prow + 64, h // 2, q0 : q0 + qlen],
                        ot[0:DK, :],
                        rb[:],
                        mybir.AluOpType.mult,
                    )

            def emit_wo(ti, copy_engine=None):
                for oc in range(2):
                    ps = wo_ps.tile([128, QCH], F32, tag="mm")
                    for j in range(2):
                        nc.tensor.matmul(
                            ps[:],
                            attnT[:, j, ti * 128 : (ti + 1) * 128],
                            woT_all[:, j, oc * QCH : (oc + 1) * QCH],
                            start=(j == 0),
                            stop=(j == 1),
                        )
                    ob = ob_pool.tile([128, QCH], F32, tag="ob")
                    if copy_engine is nc.scalar:
                        nc.scalar.copy(ob[:], ps[:])
                    else:
                        nc.vector.tensor_copy(ob[:], ps[:])
                    nc.sync.dma_start(
                        out[ti * 128 : (ti + 1) * 128, oc * QCH : (oc + 1) * QCH],
                        ob[:],
                    )

            # Pipelined emission over t-chunks: QKV/V for chunk t, then all
            # heads' attention for q-chunk t (keys/values <= t are ready),
            # then the W_O projection for the rows finished in chunk t-1.
            for tch in range(TCH):
                if tch == 0:
                    emit_qkv_chunk0()
                else:
                    for j in (0, 2, 1, 3):
                        emit_qk(j, tch)
                    for ti in range(4 * tch, 4 * tch + 4):
                        emit_v(ti)
                for h in range(HG):
                    emit_head_chunk(h, tch * QCH, QCH)
                # last chunk's psum->sbuf copies go to the otherwise idle
                # ScalarE to keep VectorE free for the normalize chains
                ce = nc.scalar if tch == TCH - 1 else None
                for ti in range(4 * tch, 4 * tch + 4):
                    emit_wo(ti, copy_engine=ce)

            if taps is not None:
                nc.sync.dma_start(taps["qkT"][:], qkT[:])
                nc.sync.dma_start(taps["vaug"][:], vaug[:])
                nc.sync.dma_start(taps["attnT"][:], attnT[:])


_CACHE = {}


def _build():
    if "nc" in _CACHE:
        return _CACHE["nc"]
    nc = bacc.Bacc("TRN2", debug=False, num_devices=N_CORES)
    xT = nc.dram_tensor("xT", [C, T], F32R, kind="ExternalInput").ap()
    wqkvT = nc.dram_tensor("wqkvT", [C, 3 * WV], F32R, kind="ExternalInput").ap()
    woT = nc.dram_tensor("woT", [2 * 128, C], F32R, kind="ExternalInput").ap()
    ones = nc.dram_tensor("ones", [128, NT, HG, 1], F32R, kind="ExternalInput").ap()
    out = nc.dram_tensor("out", [T, C], F32, kind="ExternalOutput").ap()
    _emit(nc, xT, wqkvT, woT, ones, out)
    nc.compile()
    _CACHE["nc"] = nc
    return nc


_ONES = np.ones((128, NT, HG, 1), dtype=np.float32)


def _shard_inputs(x, W_QKV, W_O):
    """Build the 8 per-core input maps. core = b*G + g."""
    in_maps = []
    W_Q, W_K, W_V = W_QKV[0:C], W_QKV[C : 2 * C], W_QKV[2 * C : 3 * C]
    for b in range(B):
        xT_b = np.ascontiguousarray(x[b].T)  # [C, T]
        for g in range(G):
            sl = slice(g * HG * DK, (g + 1) * HG * DK)
            w_g = np.concatenate([W_Q[sl], W_K[sl], W_V[sl]], axis=0)  # [768, C]
            wqkvT_g = np.ascontiguousarray(w_g.T)  # [C, 768]
            woT_g = np.ascontiguousarray(W_O[:, sl].T)  # [256, C]
            in_maps.append(
                {"xT": xT_b, "wqkvT": wqkvT_g, "woT": woT_g, "ones": _ONES}
            )
    return in_maps


def kernel(x, W_QKV, W_O):
    x = np.asarray(x, dtype=np.float32)
    W_QKV = np.asarray(W_QKV, dtype=np.float32)
    W_O = np.asarray(W_O, dtype=np.float32)
    nc = _build()
    in_maps = _shard_inputs(x, W_QKV, W_O)
    res = bass_utils.run_bass_kernel_spmd(
        nc, in_maps, core_ids=list(range(N_CORES))
    )
    out = np.zeros((B, T, C), dtype=np.float32)
    for b in range(B):
        for g in range(G):
            out[b] += res.results[b * G + g]["out"]
    return out


# revision 36
# speedup vs baseline: 1.2576x; 1.2576x over previous
"""Causal multi-head attention block (QKV proj + causal softmax attention + out proj)
for Trainium2, sharded over 8 NeuronCores.

Sharding: tensor-parallel over heads x data-parallel over batch.
  core (b, g) for b in {0,1}, g in {0..3}: batch b, head group g (4 heads of 16).
  Each core computes its 4 heads' attention output slice and a partial
  output projection (row-parallel W_O); host sums the 4 partials per batch.

Device layout: everything is computed in "transposed" orientation so no
on-chip transposes are needed anywhere:
  - host passes x^T, W_qkv^T (sliced), W_O^T (sliced) per core
  - Q^T,K^T = (W^T)^T @ x^T via PE;  V in natural [t,d] layout (+ ones column)
  - S^T[k,q] = (K^T)^T @ Q^T; exp on ScalarE (no max-subtraction needed:
    scores are ~N(0,1), exp is safe in fp32); causal mask via an additive
    -1e30 triangle on the diagonal ST psum tile before exp
  - O^T[d,q] (+ sum row from the ones column) = V_aug^T @ P^T, accumulated
    only over the causally-valid column range per k-tile
  - normalize with reciprocal_approx_fast + gpsimd partition_broadcast
  - partial_out[t,o] = (attn^T)^T @ W_O^T

All matmul operands use dtype float32r (fp32 bits, PE 'replicated' datapath):
full bf16-rate matmul at ~1e-4 relative accuracy.
"""

import sys

sys.path.insert(0, "/opt/trn_rl_repo")

import numpy as np

import concourse.bacc as bacc
import concourse.mybir as mybir
import concourse.tile as tile
from concourse import bass_utils

B, T, C = 2, 2048, 1024
H, DK = 16, 64
G = 4  # tensor-parallel head groups
HG = H // G  # heads per core
WQK = 2 * HG * DK  # 512: Q+K cols per core in wqkvT
WV = HG * DK  # 256: V cols per core
N_CORES = 8
F32 = mybir.dt.float32
F32R = mybir.dt.float32r

TCH = 4  # t chunks of 512 for N-dim of matmuls
CK = C // 128  # 8 contraction chunks
NT = T // 128  # 16 t-tiles
QCH = 512  # q chunk
MASKVAL = -1e30


def _emit(nc, xT, wqkvT, woT, ones, out, taps=None):
    with tile.TileContext(nc) as tc:
        with (
            tc.tile_pool(name="persist", bufs=1) as persist,
            tc.tile_pool(name="pt", bufs=3) as pt_pool,
            tc.tile_pool(name="small", bufs=2) as small_pool,
            tc.tile_pool(name="ob", bufs=3) as ob_pool,
            tc.tile_pool(name="qkv_ps", bufs=2, space="PSUM") as qkv_ps,
            tc.tile_pool(name="st_ps", bufs=2, space="PSUM") as st_ps,
            tc.tile_pool(name="ot_ps", bufs=2, space="PSUM") as ot_ps,
            tc.tile_pool(name="wo_ps", bufs=2, space="PSUM") as wo_ps,
        ):
            xT_all = persist.tile([128, CK, T], F32R, tag="xT_all")
            w_all = persist.tile([128, CK, 3 * WV], F32R, tag="w_all")
            woT_all = persist.tile([128, 2, C], F32R, tag="woT_all")
            qkT = persist.tile([128, 4, T], F32R, tag="qkT")
            vaug = persist.tile([128, NT, HG, DK + 1], F32R, tag="vaug")
            attnT = persist.tile([128, 2, T], F32R, tag="attnT")

            # weights + first x t-chunk interleaved, then remaining x t-chunks
            for k in range(CK):
                nc.sync.dma_start(
                    w_all[:, k, :], wqkvT[k * 128 : (k + 1) * 128, :]
                )
                nc.sync.dma_start(
                    xT_all[:, k, 0:QCH], xT[k * 128 : (k + 1) * 128, 0:QCH]
                )
            # ones column (last col) for the softmax-denominator rows
            nc.sync.dma_start(vaug[:, :, :, DK : DK + 1], ones[:])
            for tch in range(1, TCH):
                for k in range(CK):
                    nc.sync.dma_start(
                        xT_all[:, k, tch * QCH : (tch + 1) * QCH],
                        xT[k * 128 : (k + 1) * 128, tch * QCH : (tch + 1) * QCH],
                    )
            for j in range(2):
                nc.sync.dma_start(woT_all[:, j, :], woT[j * 128 : (j + 1) * 128, :])

            def qk_mm(ps, j, tch, k):
                nc.tensor.matmul(
                    ps[:],
                    w_all[:, k, j * 128 : (j + 1) * 128],
                    xT_all[:, k, tch * QCH : (tch + 1) * QCH],
                    start=(k == 0),
                    stop=(k == CK - 1),
                )

            def qk_copy(ps, j, tch):
                nc.vector.tensor_copy(qkT[:, j, tch * QCH : (tch + 1) * QCH], ps[:])

            def v_mm(ps, ti, k):
                nc.tensor.matmul(
                    ps[:],
                    xT_all[:, k, ti * 128 : (ti + 1) * 128],
                    w_all[:, k, WQK : WQK + WV],
                    start=(k == 0),
                    stop=(k == CK - 1),
                )

            def v_copy(ps, ti):
                nc.vector.tensor_copy(
                    vaug[:, ti, :, 0:DK],
                    ps[:].rearrange("p (h d) -> p h d", h=HG),
                )

            def emit_qk(j, tch):
                # qkT[:, j, t-chunk] = W[:, j*128:(j+1)*128].T @ xT
                ps = qkv_ps.tile([128, QCH], F32, tag="mm")
                for k in range(CK):
                    qk_mm(ps, j, tch, k)
                qk_copy(ps, j, tch)

            def emit_v(ti):
                # vaug[:, ti, h, 1:] = xT[:, ti-tile].T @ Wv  -> [128 t, 256]
                ps = qkv_ps.tile([128, WV], F32, tag="mm")
                for k in range(CK):
                    v_mm(ps, ti, k)
                v_copy(ps, ti)

            def emit_qkv_chunk0():
                # DMA-bound startup: k-outer over all four W tiles and the
                # first two V tiles so the PE consumes each arriving c-chunk
                # of x/w with ~6 matmuls instead of stalling per psum group.
                ps_j = {
                    j: qkv_ps.tile([128, QCH], F32, tag="mm", name=f"ps_j{j}")
                    for j in (0, 2)
                }
                ps_j[1] = ot_ps.tile([128, QCH], F32, tag="ot", name="ps_j1")
                ps_j[3] = ot_ps.tile([128, QCH], F32, tag="ot", name="ps_j3")
                ps_v = {
                    ti: st_ps.tile([128, WV], F32, tag="st", name=f"ps_v{ti}")
                    for ti in (0, 1)
                }
                for k in range(CK):
                    for j in (0, 2, 1, 3):
                        qk_mm(ps_j[j], j, 0, k)
                    for ti in (0, 1):
                        v_mm(ps_v[ti], ti, k)
                for j in (0, 2, 1, 3):
                    qk_copy(ps_j[j], j, 0)
                for ti in (0, 1):
                    v_copy(ps_v[ti], ti)
                for ti in (2, 3):
                    emit_v(ti)

            def emit_head_chunk(h, q0, qlen):
                prow = (h % 2) * 64
                QT_h = qkT[prow : prow + 64, h // 2, :]
                KT_h = qkT[prow : prow + 64, 2 + h // 2, :]
                if True:
                    nk = (q0 + qlen) // 128
                    ot = ot_ps.tile([DK + 1, QCH], F32, tag="ot", name="ot")[:, 0:qlen]
                    for k in range(nk):
                        k0 = k * 128
                        delta = k0 - q0
                        d0 = max(delta, 0)
                        st = st_ps.tile([128, QCH], F32, tag="st", name="st")[:, 0:qlen]
                        nc.tensor.matmul(
                            st[:, d0:qlen],
                            KT_h[:, k0 : k0 + 128],
                            QT_h[:, q0 + d0 : q0 + qlen],
                            start=True,
                            stop=True,
                        )
                        pt = pt_pool.tile([128, QCH], F32R, tag="pt", name="pt")[:, 0:qlen]
                        nc.scalar.activation(
                            pt[:, d0:qlen], st[:, d0:qlen],
                            mybir.ActivationFunctionType.Exp,
                            scale=float(1.0 / np.sqrt(DK)),
                        )
                        if delta >= 0:
                            # diagonal tile: zero entries with q_global < k_global
                            nc.gpsimd.affine_select(
                                out=pt[:, d0 : d0 + 128],
                                in_=pt[:, d0 : d0 + 128],
                                compare_op=mybir.AluOpType.is_ge,
                                fill=0.0,
                                base=0,
                                pattern=[[1, 128]],
                                channel_multiplier=-1,
                            )
                        # accumulate only the causally-valid columns; columns
                        # < d0 get no contribution from this k-tile (they are
                        # exactly zero), and k=0 (d0=0) initializes the bank.
                        nc.tensor.matmul(
                            ot[:, d0:qlen],
                            vaug[:, k, h, :],
                            pt[:, d0:qlen],
                            start=(k == 0),
                            stop=(k == nk - 1),
                        )
                    # custom-DVE recip needs a base-partition-0 source; stage
                    # the sums row (psum partition 64) through sbuf partition 0
                    sums_sb = small_pool.tile([1, QCH], F32, tag="sums", name="sums_sb")[:, 0:qlen]
                    nc.vector.tensor_copy(sums_sb[:], ot[DK : DK + 1, :])
                    recip = small_pool.tile([1, QCH], F32, tag="recip", name="recip")[:, 0:qlen]
                    nc.vector.reciprocal_approx_fast(out=recip[:], in_=sums_sb[:])
                    rb = small_pool.tile([64, QCH], F32, tag="rb", name="rb")[:, 0:qlen]
                    nc.gpsimd.partition_broadcast(rb[:], recip[:])
                    nc.vector.tensor_tensor(
                        attnT[prow : prow + 64, h // 2, q0 : q0 + qlen],
                        ot[0:DK, :],
                        rb[:],
                        mybir.AluOpType.mult,
                    )

            def emit_wo(ti, copy_engine=None):
                for oc in range(2):
                    ps = wo_ps.tile([128, QCH], F32, tag="wo")
                    for j in range(2):
                        nc.tensor.matmul(
                            ps[:],
                            attnT[:, j, ti * 128 : (ti + 1) * 128],
                            woT_all[:, j, oc * QCH : (oc + 1) * QCH],
                            start=(j == 0),
                            stop=(j == 1),
                        )
                    ob = ob_pool.tile([128, QCH], F32, tag="ob")
                    if copy_engine is nc.scalar:
                        nc.scalar.copy(ob[:], ps[:])
                    else:
                        nc.vector.tensor_copy(ob[:], ps[:])
                    nc.sync.dma_start(
                        out[ti * 128 : (ti + 1) * 128, oc * QCH : (oc + 1) * QCH],
                        ob[:],
                    )

            # Pipelined emission over t-chunks: QKV/V for chunk t, then all
            # heads' attention for q-chunk t (keys/values <= t are ready),
            # then the W_O projection for the rows finished in chunk t-1.
            for tch in range(TCH):
                if tch == 0:
                    emit_qkv_chunk0()
                else:
                    for j in (0, 2, 1, 3):
                        emit_qk(j, tch)
                    for ti in range(4 * tch, 4 * tch + 4):
                        emit_v(ti)
                for h in range(HG):
                    emit_head_chunk(h, tch * QCH, QCH)
                # last chunk's psum->sbuf copies go to the otherwise idle
                # ScalarE to keep VectorE free for the normalize chains
                ce = nc.scalar if tch == TCH - 1 else None
                for ti in range(4 * tch, 4 * tch + 4):
                    emit_wo(ti, copy_engine=ce)

            if taps is not None:
                nc.sync.dma_start(taps["qkT"][:], qkT[:])
                nc.sync.dma_start(taps["vaug"][:], vaug[:])
                nc.sync.dma_start(taps["attnT"][:], attnT[:])


_CACHE = {}


def _build():
    if "nc" in _CACHE:
        return _CACHE["nc"]
    nc = bacc.Bacc("TRN2", debug=False, num_devices=N_CORES)
    xT = nc.dram_tensor("xT", [C, T], F32R, kind="ExternalInput").ap()
    wqkvT = nc.dram_tensor("wqkvT", [C, 3 * WV], F32R, kind="ExternalInput").ap()
    woT = nc.dram_tensor("woT", [2 * 128, C], F32R, kind="ExternalInput").ap()
    ones = nc.dram_tensor("ones", [128, NT, HG, 1], F32R, kind="ExternalInput").ap()
    out = nc.dram_tensor("out", [T, C], F32, kind="ExternalOutput").ap()
    _emit(nc, xT, wqkvT, woT, ones, out)
    nc.compile()
    _CACHE["nc"] = nc
    return nc


_ONES = np.ones((128, NT, HG, 1), dtype=np.float32)


def _shard_inputs(x, W_QKV, W_O):
    """Build the 8 per-core input maps. core = b*G + g."""
    in_maps = []
    W_Q, W_K, W_V = W_QKV[0:C], W_QKV[C : 2 * C], W_QKV[2 * C : 3 * C]
    for b in range(B):
        xT_b = np.ascontiguousarray(x[b].T)  # [C, T]
        for g in range(G):
            sl = slice(g * HG * DK, (g + 1) * HG * DK)
            w_g = np.concatenate([W_Q[sl], W_K[sl], W_V[sl]], axis=0)  # [768, C]
            wqkvT_g = np.ascontiguousarray(w_g.T)  # [C, 768]
            woT_g = np.ascontiguousarray(W_O[:, sl].T)  # [256, C]
            in_maps.append(
                {"xT": xT_b, "wqkvT": wqkvT_g, "woT": woT_g, "ones": _ONES}
            )
    return in_maps


def kernel(x, W_QKV, W_O):
    x = np.asarray(x, dtype=np.float32)
    W_QKV = np.asarray(W_QKV, dtype=np.float32)
    W_O = np.asarray(W_O, dtype=np.float32)
    nc = _build()
    in_maps = _shard_inputs(x, W_QKV, W_O)
    res = bass_utils.run_bass_kernel_spmd(
        nc, in_maps, core_ids=list(range(N_CORES))
    )
    out = np.zeros((B, T, C), dtype=np.float32)
    for b in range(B):
        for g in range(G):
            out[b] += res.results[b * G + g]["out"]
    return out


# revision 53
# speedup vs baseline: 1.3796x; 1.0970x over previous
"""Causal multi-head attention block (QKV proj + causal softmax attention + out proj)
for Trainium2, sharded over 8 NeuronCores.

Sharding: tensor-parallel over heads x data-parallel over batch.
  core (b, g) for b in {0,1}, g in {0..3}: batch b, head group g (4 heads of 16).
  Each core computes its 4 heads' attention output slice and a partial
  output projection (row-parallel W_O); host sums the 4 partials per batch.

Device layout: everything is computed in "transposed" orientation so no
on-chip transposes are needed anywhere:
  - host passes x^T, W_qkv^T (sliced), W_O^T (sliced) per core
  - Q^T,K^T = (W^T)^T @ x^T via PE;  V in natural [t,d] layout (+ ones column)
  - S^T[k,q] = (K^T)^T @ Q^T; exp on ScalarE (no max-subtraction needed:
    scores are ~N(0,1), exp is safe in fp32); causal mask via an additive
    -1e30 triangle on the diagonal ST psum tile before exp
  - O^T[d,q] (+ sum row from the ones column) = V_aug^T @ P^T, accumulated
    only over the causally-valid column range per k-tile
  - normalize with reciprocal_approx_fast + gpsimd partition_broadcast
  - partial_out[t,o] = (attn^T)^T @ W_O^T

All matmul operands use dtype float32r (fp32 bits, PE 'replicated' datapath):
full bf16-rate matmul at ~1e-4 relative accuracy.
"""

import sys

sys.path.insert(0, "/opt/trn_rl_repo")

import numpy as np

import concourse.bacc as bacc
import concourse.mybir as mybir
import concourse.tile as tile
from concourse import bass_utils

B, T, C = 2, 2048, 1024
H, DK = 16, 64
G = 4  # tensor-parallel head groups
HG = H // G  # heads per core
WQK = 2 * HG * DK  # 512: Q+K cols per core in wqkvT
WV = HG * DK  # 256: V cols per core
N_CORES = 8
F32 = mybir.dt.float32
F32R = mybir.dt.float32r

TCH = 4  # t chunks of 512 for N-dim of matmuls
CK = C // 128  # 8 contraction chunks
NT = T // 128  # 16 t-tiles
QCH = 512  # q chunk
MASKVAL = -1e30


def _emit(nc, xT, wqkvT, woT, ones, out, taps=None):
    with tile.TileContext(nc) as tc:
        with (
            tc.tile_pool(name="persist", bufs=1) as persist,
            tc.tile_pool(name="pt", bufs=4) as pt_pool,
            tc.tile_pool(name="small", bufs=4) as small_pool,
            tc.tile_pool(name="ob", bufs=6) as ob_pool,
            tc.tile_pool(name="qkv_ps", bufs=2, space="PSUM") as qkv_ps,
            tc.tile_pool(name="st_ps", bufs=3, space="PSUM") as st_ps,
            tc.tile_pool(name="ot_ps", bufs=2, space="PSUM") as ot_ps,
            tc.tile_pool(name="wo_ps", bufs=1, space="PSUM") as wo_ps,
        ):
            xT_all = persist.tile([128, CK, T], F32R, tag="xT_all")
            w_all = persist.tile([128, CK, 3 * WV], F32R, tag="w_all")
            woT_all = persist.tile([128, 2, C], F32R, tag="woT_all")
            qkT = persist.tile([128, 4, T], F32R, tag="qkT")
            vaug = persist.tile([128, NT, HG, DK + 1], F32R, tag="vaug")
            attnT = persist.tile([128, 2, T], F32R, tag="attnT")

            # weights + first x t-chunk interleaved, then remaining x t-chunks;
            # w and x go through different engines' DMA queues so the first
            # c-chunks arrive in parallel
            for k in range(CK):
                nc.sync.dma_start(
                    w_all[:, k, :], wqkvT[k * 128 : (k + 1) * 128, :]
                )
                nc.scalar.dma_start(
                    xT_all[:, k, 0:QCH], xT[k * 128 : (k + 1) * 128, 0:QCH]
                )
            # ones column (last col) for the softmax-denominator rows
            nc.sync.dma_start(vaug[:, :, :, DK : DK + 1], ones[:])
            for tch in range(1, TCH):
                for k in range(CK):
                    nc.sync.dma_start(
                        xT_all[:, k, tch * QCH : (tch + 1) * QCH],
                        xT[k * 128 : (k + 1) * 128, tch * QCH : (tch + 1) * QCH],
                    )
            for j in range(2):
                nc.sync.dma_start(woT_all[:, j, :], woT[j * 128 : (j + 1) * 128, :])

            def qk_mm(ps, j, tch, k):
                nc.tensor.matmul(
                    ps[:],
                    w_all[:, k, j * 128 : (j + 1) * 128],
                    xT_all[:, k, tch * QCH : (tch + 1) * QCH],
                    start=(k == 0),
                    stop=(k == CK - 1),
                )

            def qk_copy(ps, j, tch):
                nc.vector.tensor_copy(qkT[:, j, tch * QCH : (tch + 1) * QCH], ps[:])

            def v_mm(ps, ti, k):
                nc.tensor.matmul(
                    ps[:],
                    xT_all[:, k, ti * 128 : (ti + 1) * 128],
                    w_all[:, k, WQK : WQK + WV],
                    start=(k == 0),
                    stop=(k == CK - 1),
                )

            def v_copy(ps, ti):
                nc.vector.tensor_copy(
                    vaug[:, ti, :, 0:DK],
                    ps[:].rearrange("p (h d) -> p h d", h=HG),
                )

            def emit_qk(j, tch):
                # qkT[:, j, t-chunk] = W[:, j*128:(j+1)*128].T @ xT
                ps = qkv_ps.tile([128, QCH], F32, tag="mm")
                for k in range(CK):
                    qk_mm(ps, j, tch, k)
                qk_copy(ps, j, tch)

            def emit_v(ti):
                # vaug[:, ti, h, 1:] = xT[:, ti-tile].T @ Wv  -> [128 t, 256]
                ps = qkv_ps.tile([128, WV], F32, tag="mm")
                for k in range(CK):
                    v_mm(ps, ti, k)
                v_copy(ps, ti)

            def emit_qkv_chunk0():
                # DMA-bound startup: k-outer over all four W tiles and the
                # first two V tiles so the PE consumes each arriving c-chunk
                # of x/w with ~6 matmuls instead of stalling per psum group.
                ps_j = {
                    j: qkv_ps.tile([128, QCH], F32, tag="mm", name=f"ps_j{j}")
                    for j in (0, 2)
                }
                ps_j[1] = ot_ps.tile([128, QCH], F32, tag="ot", name="ps_j1")
                ps_j[3] = ot_ps.tile([128, QCH], F32, tag="ot", name="ps_j3")
                ps_v = {
                    ti: st_ps.tile([128, WV], F32, tag="st", name=f"ps_v{ti}")
                    for ti in (0, 1)
                }
                for k in range(CK):
                    for j in (0, 2, 1, 3):
                        qk_mm(ps_j[j], j, 0, k)
                    for ti in (0, 1):
                        v_mm(ps_v[ti], ti, k)
                for j in (0, 2, 1, 3):
                    qk_copy(ps_j[j], j, 0)
                for ti in (0, 1):
                    v_copy(ps_v[ti], ti)
                for ti in (2, 3):
                    emit_v(ti)

            def emit_head_chunk(h, q0, qlen):
                prow = (h % 2) * 64
                QT_h = qkT[prow : prow + 64, h // 2, :]
                KT_h = qkT[prow : prow + 64, 2 + h // 2, :]
                if True:
                    nk = (q0 + qlen) // 128
                    ot = ot_ps.tile([DK + 1, QCH], F32, tag="ot", name="ot")[:, 0:qlen]
                    for k in range(nk):
                        k0 = k * 128
                        delta = k0 - q0
                        d0 = max(delta, 0)
                        st = st_ps.tile([128, QCH], F32, tag="st", name="st")[:, 0:qlen]
                        nc.tensor.matmul(
                            st[:, d0:qlen],
                            KT_h[:, k0 : k0 + 128],
                            QT_h[:, q0 + d0 : q0 + qlen],
                            start=True,
                            stop=True,
                        )
                        pt = pt_pool.tile([128, QCH], F32R, tag="pt", name="pt")[:, 0:qlen]
                        nc.scalar.activation(
                            pt[:, d0:qlen], st[:, d0:qlen],
                            mybir.ActivationFunctionType.Exp,
                            scale=float(1.0 / np.sqrt(DK)),
                        )
                        if delta >= 0:
                            # diagonal tile: zero entries with q_global < k_global
                            nc.gpsimd.affine_select(
                                out=pt[:, d0 : d0 + 128],
                                in_=pt[:, d0 : d0 + 128],
                                compare_op=mybir.AluOpType.is_ge,
                                fill=0.0,
                                base=0,
                                pattern=[[1, 128]],
                                channel_multiplier=-1,
                            )
                        # accumulate only the causally-valid columns; columns
                        # < d0 get no contribution from this k-tile (they are
                        # exactly zero), and k=0 (d0=0) initializes the bank.
                        nc.tensor.matmul(
                            ot[:, d0:qlen],
                            vaug[:, k, h, :],
                            pt[:, d0:qlen],
                            start=(k == 0),
                            stop=(k == nk - 1),
                        )
                    # custom-DVE recip needs a base-partition-0 source; stage
                    # the sums row (psum partition 64) through sbuf partition 0
                    sums_sb = small_pool.tile([1, QCH], F32, tag="sums", name="sums_sb")[:, 0:qlen]
                    nc.vector.tensor_copy(sums_sb[:], ot[DK : DK + 1, :])
                    recip = small_pool.tile([1, QCH], F32, tag="recip", name="recip")[:, 0:qlen]
                    nc.vector.reciprocal_approx_fast(out=recip[:], in_=sums_sb[:])
                    rb = small_pool.tile([64, QCH], F32, tag="rb", name="rb")[:, 0:qlen]
                    nc.gpsimd.partition_broadcast(rb[:], recip[:])
                    nc.vector.tensor_tensor(
                        attnT[prow : prow + 64, h // 2, q0 : q0 + qlen],
                        ot[0:DK, :],
                        rb[:],
                        mybir.AluOpType.mult,
                    )

            def emit_wo(ti, alternate=False, pool=None, ptag="wo"):
                for oc in range(2):
                    ps = (pool or wo_ps).tile([128, QCH], F32, tag=ptag, name="wops")
                    for j in range(2):
                        nc.tensor.matmul(
                            ps[:],
                            attnT[:, j, ti * 128 : (ti + 1) * 128],
                            woT_all[:, j, oc * QCH : (oc + 1) * QCH],
                            start=(j == 0),
                            stop=(j == 1),
                        )
                    ob = ob_pool.tile([128, QCH], F32, tag="ob")
                    if alternate and (ti + oc) % 2 == 0:
                        nc.scalar.copy(ob[:], ps[:])
                    else:
                        nc.vector.tensor_copy(ob[:], ps[:])
                    nc.sync.dma_start(
                        out[ti * 128 : (ti + 1) * 128, oc * QCH : (oc + 1) * QCH],
                        ob[:],
                    )

            # Pipelined emission over t-chunks. Attention segments are paced
            # by ScalarE's exp, so each segment's head chunks are interleaved
            # with dense PE filler work: the NEXT chunk's QKV/V projections
            # and the PREVIOUS chunk's W_O tiles.
            emit_qkv_chunk0()
            for seg in range(TCH):
                fillers = []
                nxt = seg + 1
                if nxt < TCH:
                    for j in (0, 2, 1, 3):
                        fillers.append(lambda j=j: emit_qk(j, nxt))
                    for ti in range(4 * nxt, 4 * nxt + 4):
                        fillers.append(lambda ti=ti: emit_v(ti))
                if seg >= 1:
                    for ti in range(4 * (seg - 1), 4 * (seg - 1) + 4):
                        fillers.append(lambda ti=ti: emit_wo(ti))
                per = (len(fillers) + HG - 1) // HG if fillers else 0
                for h in range(HG):
                    emit_head_chunk(h, seg * QCH, QCH)
                    for f in fillers[h * per : (h + 1) * per]:
                        f()
            # tail: the last chunk's W_O through the now-idle 2-slot qkv pool,
            # copies alternating between ScalarE and VectorE
            for ti in range(4 * (TCH - 1), 4 * (TCH - 1) + 4):
                emit_wo(ti, alternate=True, pool=qkv_ps, ptag="mm")

            if taps is not None:
                nc.sync.dma_start(taps["qkT"][:], qkT[:])
                nc.sync.dma_start(taps["vaug"][:], vaug[:])
                nc.sync.dma_start(taps["attnT"][:], attnT[:])


_CACHE = {}


def _build():
    if "nc" in _CACHE:
        return _CACHE["nc"]
    nc = bacc.Bacc("TRN2", debug=False, num_devices=N_CORES)
    xT = nc.dram_tensor("xT", [C, T], F32R, kind="ExternalInput").ap()
    wqkvT = nc.dram_tensor("wqkvT", [C, 3 * WV], F32R, kind="ExternalInput").ap()
    woT = nc.dram_tensor("woT", [2 * 128, C], F32R, kind="ExternalInput").ap()
    ones = nc.dram_tensor("ones", [128, NT, HG, 1], F32R, kind="ExternalInput").ap()
    out = nc.dram_tensor("out", [T, C], F32, kind="ExternalOutput").ap()
    _emit(nc, xT, wqkvT, woT, ones, out)
    nc.compile()
    _CACHE["nc"] = nc
    return nc


_ONES = np.ones((128, NT, HG, 1), dtype=np.float32)


def _shard_inputs(x, W_QKV, W_O):
    """Build the 8 per-core input maps. core = b*G + g."""
    in_maps = []
    W_Q, W_K, W_V = W_QKV[0:C], W_QKV[C : 2 * C], W_QKV[2 * C : 3 * C]
    for b in range(B):
        xT_b = np.ascontiguousarray(x[b].T)  # [C, T]
        for g in range(G):
            sl = slice(g * HG * DK, (g + 1) * HG * DK)
            w_g = np.concatenate([W_Q[sl], W_K[sl], W_V[sl]], axis=0)  # [768, C]
            wqkvT_g = np.ascontiguousarray(w_g.T)  # [C, 768]
            woT_g = np.ascontiguousarray(W_O[:, sl].T)  # [256, C]
            in_maps.append(
                {"xT": xT_b, "wqkvT": wqkvT_g, "woT": woT_g, "ones": _ONES}
            )
    return in_maps


def kernel(x, W_QKV, W_O):
    x = np.asarray(x, dtype=np.float32)
    W_QKV = np.asarray(W_QKV, dtype=np.float32)
    W_O = np.asarray(W_O, dtype=np.float32)
    nc = _build()
    in_maps = _shard_inputs(x, W_QKV, W_O)
    res = bass_utils.run_bass_kernel_spmd(
        nc, in_maps, core_ids=list(range(N_CORES))
    )
    out = np.zeros((B, T, C), dtype=np.float32)
    for b in range(B):
        for g in range(G):
            out[b] += res.results[b * G + g]["out"]
    return out
